# revision 54
# baseline (speedup 1.0000x reference)
"""Trainium2 Bass kernel for nn_AttentionWithTime.

Data-parallel over batch: B=8 batches -> 8 NeuronCores, one batch each.
Per-core strategy: all sequence-dim normalization runs in transposed
([feature, seq]) orientation so it is a free-dim reduction; the
Gaussian-kernel attention distance matrix d2 = q2 + k2 - 2qk is produced by
a single K=97 augmented f32r matmul (k rows | k2 | zero pad | ones against
-2q rows | ones | zero pad | q2 -- aug rows sit at partitions 64/96 because
engine partition offsets must be multiples of 32), exp'd in one ScalarE
pass from PSUM; attention output accumulates head-transposed, is merged
through Wm in natural orientation only, and x1^T is recovered with 16 PE
transposes whose PSUM->SBUF copies fuse the bm + x residual.  Small
per-feature vectors ship as one packed [40,128] DMA + a single PE
transpose.  All matmuls run as float32r (full PE rate at free>=256).
"""

import numpy as np

B, N, D = 8, 1024, 256
H, DH = 8, 64
DE, DT = 1024, 256
DQ = H * DH            # 512
DV = D * H             # 2048
DQKV = 2 * DQ + DV     # 3072
EPS = 1e-5
NCORES = 8

_CACHE = {}
SIM_COMPAT = False  # CoreSim lacks Silu; decompose when simulating


def _build():
    import concourse.bass as bass
    import concourse.mybir as mybir
    import concourse.tile as tile
    from concourse import bacc
    from concourse.masks import make_identity

    F32 = mybir.dt.float32
    F32R = mybir.dt.float32r
    AF = mybir.ActivationFunctionType
    OP = mybir.AluOpType

    nc = bacc.Bacc()

    xT_d = nc.declare_dram_parameter("xT", [D, N], F32, isOutput=False)
    xn_d = nc.declare_dram_parameter("xnat", [N, D], F32, isOutput=False)
    posT_d = nc.declare_dram_parameter("posT", [D, N], F32, isOutput=False)
    wqkv_d = nc.declare_dram_parameter("Wqkv", [D, DQKV], F32, isOutput=False)
    wm_d = nc.declare_dram_parameter("Wm", [DV, D], F32, isOutput=False)
    wf1_d = nc.declare_dram_parameter("Wf1", [D, DE], F32, isOutput=False)
    wf2_d = nc.declare_dram_parameter("Wf2", [DE, D], F32, isOutput=False)
    wt_d = nc.declare_dram_parameter("Wt", [DT, D + DE], F32, isOutput=False)
    bm_d = nc.declare_dram_parameter("bm", [D], F32, isOutput=False)
    bqkv_d = nc.declare_dram_parameter("bqkv", [DQKV], F32, isOutput=False)
    bf2_d = nc.declare_dram_parameter("bf2", [D], F32, isOutput=False)
    vp_d = nc.declare_dram_parameter("vecpack", [40, 128], F32, isOutput=False)
    out_d = nc.declare_dram_parameter("out", [N, D], F32, isOutput=True)

    def r(ap):
        return ap.bitcast(F32R)

    with tile.TileContext(nc) as tc:
        with (
            tc.tile_pool(name="L1", bufs=1) as L1,
            tc.tile_pool(name="ps_s", bufs=3, space="PSUM") as ps_s,
            tc.tile_pool(name="ps_av", bufs=3, space="PSUM") as ps_av,
            tc.tile_pool(name="ps_v", bufs=2, space="PSUM") as ps_v,
        ):
            # ---- long-lived SBUF ----
            xT = L1.tile([128, 2, N], F32)
            hT = L1.tile([128, 2, N], F32)
            accT = L1.tile([128, 2, N], F32)
            accN = L1.tile([128, 8, D], F32)
            wm = L1.tile([128, 16, D], F32)
            for dc in range(2):
                nc.sync.dma_start(xT[:, dc, :], xT_d[dc * 128:(dc + 1) * 128, :])
            # small per-feature vectors arrive packed as [40,128]; one PE
            # transpose turns them into per-partition columns.
            vp_s = L1.tile([40, 128], F32)
            nc.sync.dma_start(vp_s[:], vp_d[:])
            ident = L1.tile([128, 128], F32)
            make_identity(nc, ident[:])
            vp = L1.tile([128, 40], F32)
            pvp = ps_v.tile([128, 40], F32, tag="v")
            nc.tensor.transpose(pvp[:], vp_s[:], ident[0:40, 0:40])
            nc.vector.tensor_copy(vp[:], pvp[:])
            # only the t-vector columns feed a matmul -> separate f32r tile
            tvec_t = L1.tile([128, 3], F32)
            nc.vector.tensor_copy(tvec_t[:].bitcast(F32R), pvp[:, 10:13])

            # vecpack column map (see _make_in_maps): gamma 0:2, beta 2:4,
            # bm 4:6, gamma1 6:8, beta1 8:10, tvec 10:12 (+zero col 12),
            # bt 13:23, bf1 23:31, bqkv q/k chunks 31:39, expscale 39
            gamma_t = vp[:, 0:2]
            beta_t = vp[:, 2:4]
            bm_t = vp[:, 4:6]
            gamma1_t = vp[:, 6:8]
            beta1_t = vp[:, 8:10]
            bt_t = vp[:, 13:23]
            bf1_t = vp[:, 23:31]
            bqkv_q = vp[:, 31:39]
            esc_t = vp[:, 39:40]
            bm_bc = L1.tile([128, D], F32)
            bf2_bc = L1.tile([128, D], F32)
            eps_t = L1.tile([128, 1], F32)
            nc.vector.memset(eps_t[:], EPS)
            # f32r cannot be memset directly; stage in f32 then cast-copy
            oS = L1.tile([128, 2], F32)
            nc.vector.memset(oS[:], 0.0)
            nc.vector.memset(oS[0:64, 0:1], 1.0)
            nc.vector.memset(oS[64:128, 1:2], 1.0)
            onesA = L1.tile([128, 1], F32)
            nc.vector.tensor_copy(onesA[:].bitcast(F32R), oS[:, 0:1])
            onesB = L1.tile([128, 1], F32)
            nc.vector.tensor_copy(onesB[:].bitcast(F32R), oS[:, 1:2])
            # staging rows for augmented tiles: kq_c = [1; 0*32], kk_c = [0*32; 1]
            kq_c = L1.tile([33, N], F32)
            nc.vector.memset(kq_c[0:33, :], 0.0)
            nc.vector.memset(kq_c[0:1, :], 1.0)
            kk_c = L1.tile([33, N], F32)
            nc.vector.memset(kk_c[0:33, :], 0.0)
            nc.vector.memset(kk_c[32:33, :], 1.0)
            ts_all = L1.tile([128, 10], F32)
            sig_all = L1.tile([128, 8], F32)
            bf1sig = L1.tile([128, 8], F32)
            stat = L1.tile([128, 2, 2, nc.vector.BN_STATS_DIM], F32)
            mv = L1.tile([128, 2, nc.vector.BN_AGGR_DIM], F32)
            nA = L1.tile([128, 2], F32)
            nB = L1.tile([128, 2], F32)
            tmp1 = L1.tile([128, 2], F32)

            if SIM_COMPAT:
                sil_t = L1.tile([128, N], F32, tag="sil")
            else:
                sil_t = None

            def act_silu(out_ap, in_ap, bias=0.0, scale=1.0, width=N):
                # out = Silu(in*scale + bias); decomposed for CoreSim
                if not SIM_COMPAT:
                    nc.scalar.activation(out_ap, in_ap, AF.Silu,
                                         bias=bias, scale=scale)
                    return
                sg = sil_t
                nc.scalar.activation(sg[:, :width], in_ap, AF.Sigmoid,
                                     bias=bias, scale=scale)
                if isinstance(scale, float) and isinstance(bias, float):
                    if scale == 1.0 and bias == 0.0:
                        nc.vector.tensor_mul(out_ap, in_ap, sg[:, :width])
                        return
                    raise NotImplementedError
                nc.vector.tensor_scalar(out_ap, in_ap, scale, bias,
                                        OP.mult, OP.add)
                nc.vector.tensor_mul(out_ap, out_ap, sg[:, :width])

            def norm_affine(src_ap_fn, g_t, b_t, dc):
                # src [128, N] in transposed layout; per-partition affine out
                nc.vector.bn_stats(stat[:, dc, 0, :], src_ap_fn()[:, 0:512])
                nc.vector.bn_stats(stat[:, dc, 1, :], src_ap_fn()[:, 512:1024])
                nc.vector.bn_aggr(mv[:, dc, :], stat[:, dc, :, :])
                nc.scalar.activation(tmp1[:, dc:dc + 1], mv[:, dc, 1:2],
                                     AF.Sqrt, bias=eps_t[:], scale=1.0)
                nc.vector.reciprocal(nA[:, dc:dc + 1], tmp1[:, dc:dc + 1])
                nc.vector.tensor_mul(nA[:, dc:dc + 1], nA[:, dc:dc + 1],
                                     g_t[:, dc:dc + 1])
                nc.vector.tensor_mul(tmp1[:, dc:dc + 1], mv[:, dc, 0:1],
                                     nA[:, dc:dc + 1])
                nc.vector.tensor_tensor(nB[:, dc:dc + 1], b_t[:, dc:dc + 1],
                                        tmp1[:, dc:dc + 1], OP.subtract)

            with tc.tile_pool(name="L2", bufs=1) as L2:
                # load order matters: q/k weight columns + posT gate the
                # prologue; v columns, Wm and broadcast consts come later.
                wqkv = L2.tile([128, 2, DQKV], F32)
                posT = L2.tile([128, 2, N], F32)
                wt = L2.tile([128, 2, D + DE], F32)
                bqkv_vbc = L2.tile([128, DV], F32)
                for dc in range(2):
                    nc.sync.dma_start(
                        wqkv[:, dc, 0:2 * DQ].bitcast(F32R),
                        wqkv_d[dc * 128:(dc + 1) * 128, 0:2 * DQ].bitcast(F32R))
                for dc in range(2):
                    nc.sync.dma_start(posT[:, dc, :],
                                        posT_d[dc * 128:(dc + 1) * 128, :])
                for dc in range(2):
                    nc.scalar.dma_start(
                        wt[:, dc, :].bitcast(F32R),
                        wt_d[dc * 128:(dc + 1) * 128, :].bitcast(F32R))
                for dc in range(2):
                    nc.sync.dma_start(
                        wqkv[:, dc, 2 * DQ:].bitcast(F32R),
                        wqkv_d[dc * 128:(dc + 1) * 128, 2 * DQ:].bitcast(F32R))
                nc.scalar.dma_start(bqkv_vbc[:],
                                    bqkv_d[2 * DQ:].partition_broadcast(128))
                for fc in range(16):
                    nc.sync.dma_start(
                        wm[:, fc, :].bitcast(F32R),
                        wm_d[fc * 128:(fc + 1) * 128, :].bitcast(F32R))
                nc.scalar.dma_start(bm_bc[:], bm_d[:].partition_broadcast(128))
                nc.scalar.dma_start(bf2_bc[:], bf2_d[:].partition_broadcast(128))

                # ---- phase A: feature_norm + pos_enc + swish -> hT ----
                for dc in range(2):
                    norm_affine(lambda: xT[:, dc, :], gamma_t, beta_t, dc)
                    for nh in range(2):
                        ns = slice(nh * 512, nh * 512 + 512)
                        nc.vector.tensor_scalar(hT[:, dc, ns].bitcast(F32R),
                                                xT[:, dc, ns],
                                                nA[:, dc:dc + 1],
                                                nB[:, dc:dc + 1],
                                                OP.mult, OP.add)
                        nc.vector.tensor_tensor(hT[:, dc, ns].bitcast(F32R),
                                                hT[:, dc, ns],
                                                posT[:, dc, ns], OP.add)
                        act_silu(hT[:, dc, ns].bitcast(F32R), hT[:, dc, ns],
                                 width=512)

                # ---- phase A2: time branch ts = t @ Wt + bt ----
                for j in range(10):
                    # fp32r matmul needs free>=2: col 1 accumulates junk
                    # (t1-projection then +0), only col 0 is consumed.
                    pts = ps_v.tile([128, 2], F32, tag="v")
                    for dc in range(2):
                        nc.tensor.matmul(pts[:], r(wt[:, dc, j * 128:(j + 1) * 128]),
                                         r(tvec_t[:, dc:dc + 2]),
                                         start=(dc == 0), stop=(dc == 1))
                    nc.vector.tensor_tensor(ts_all[:, j:j + 1], pts[:, 0:1],
                                            bt_t[:, j:j + 1], OP.add)
                for j in range(8):
                    nc.scalar.activation(sig_all[:, j:j + 1],
                                         ts_all[:, j + 2:j + 3], AF.Sigmoid)
                    nc.vector.tensor_mul(bf1sig[:, j:j + 1], sig_all[:, j:j + 1],
                                         bf1_t[:, j:j + 1])

                with (
                    tc.tile_pool(name="qa_p", bufs=4) as qa_p,
                    tc.tile_pool(name="ka_p", bufs=4) as ka_p,
                    tc.tile_pool(name="sq_p", bufs=2) as sq_p,
                    tc.tile_pool(name="vh_p", bufs=2) as vh_p,
                    tc.tile_pool(name="ex_p", bufs=4) as ex_p,
                    tc.tile_pool(name="oT_p", bufs=1) as oT_p,
                ):
                    qa_tiles, ka_tiles = {}, {}

                    def project_qk(j):
                        # heads 2j, 2j+1 -> augmented q/k tiles.
                        # aug layout (rows): 0:64 data, 64 = ones(q)/k2(k),
                        # 65:96 zero, 96 = q2(q)/ones(k); engine partition
                        # offsets must be multiples of 32.
                        h0, h1 = 2 * j, 2 * j + 1
                        qa0 = qa_p.tile([97, N], F32, tag="qa")
                        qa1 = qa_p.tile([97, N], F32, tag="qa")
                        ka0 = ka_p.tile([97, N], F32, tag="ka")
                        ka1 = ka_p.tile([97, N], F32, tag="ka")
                        qa_tiles[h0], qa_tiles[h1] = qa0, qa1
                        ka_tiles[h0], ka_tiles[h1] = ka0, ka1
                        for qa in (qa0, qa1):
                            nc.gpsimd.tensor_copy(qa[64:97, :].bitcast(F32R),
                                                  kq_c[0:33, :])
                        for ka in (ka0, ka1):
                            nc.gpsimd.tensor_copy(ka[64:97, :].bitcast(F32R),
                                                  kk_c[0:33, :])
                        for nh in range(2):
                            ns = slice(nh * 512, nh * 512 + 512)
                            pq = ps_s.tile([128, 512], F32, tag="s")
                            pk = ps_s.tile([128, 512], F32, tag="s")
                            for dc in range(2):
                                nc.tensor.matmul(
                                    pq[:], r(wqkv[:, dc, j * 128:(j + 1) * 128]),
                                    r(hT[:, dc, ns]),
                                    start=(dc == 0), stop=(dc == 1))
                            for dc in range(2):
                                nc.tensor.matmul(
                                    pk[:], r(wqkv[:, dc, DQ + j * 128:DQ + (j + 1) * 128]),
                                    r(hT[:, dc, ns]),
                                    start=(dc == 0), stop=(dc == 1))
                            # q rows: -2*(q+bias); k rows: k+bias; biases
                            # folded into the copies and the squares
                            bq, bk = bqkv_q[:, j:j + 1], bqkv_q[:, 4 + j:5 + j]
                            nc.vector.tensor_scalar(
                                qa0[0:64, ns].bitcast(F32R), pq[0:64, :],
                                bq[0:64, :], -2.0, OP.add, OP.mult)
                            nc.vector.tensor_scalar(
                                qa1[0:64, ns].bitcast(F32R), pq[64:128, :],
                                bq[64:128, :], -2.0, OP.add, OP.mult)
                            nc.vector.tensor_scalar(
                                ka0[0:64, ns].bitcast(F32R), pk[0:64, :],
                                bk[0:64, :], None, OP.add)
                            nc.vector.tensor_scalar(
                                ka1[0:64, ns].bitcast(F32R), pk[64:128, :],
                                bk[64:128, :], None, OP.add)
                            sqq = sq_p.tile([128, 512], F32, tag="sq")
                            sqk = sq_p.tile([128, 512], F32, tag="sq")
                            nc.scalar.activation(sqq[:].bitcast(F32R), pq[:],
                                                 AF.Square, bias=bq)
                            nc.scalar.activation(sqk[:].bitcast(F32R), pk[:],
                                                 AF.Square, bias=bk)
                            for (aug, row, ones_v, sq_t) in (
                                (qa0, 96, onesA, sqq), (qa1, 96, onesB, sqq),
                                (ka0, 64, onesA, sqk), (ka1, 64, onesB, sqk),
                            ):
                                p2 = ps_av.tile([1, 512], F32, tag="av")
                                nc.tensor.matmul(p2[:], r(ones_v[:]), r(sq_t[:]),
                                                 start=True, stop=True)
                                nc.scalar.activation(
                                    aug[row:row + 1, ns].bitcast(F32R), p2[0:1, :],
                                    AF.Copy)

                    # ---- attention + split merge, two groups of 4 heads;
                    # projections prefetched one head-pair ahead ----
                    project_qk(0)
                    for g in range(2):
                        outT = oT_p.tile([128, 8, N], F32, tag="outT")
                        for hh in range(4):
                            h = 4 * g + hh
                            if h % 2 == 0 and h < 6:
                                project_qk(h // 2 + 1)
                            vh = vh_p.tile([128, 8, D], F32, tag="vh")
                            for mc in range(8):
                                pv = ps_v.tile([128, D], F32, tag="v")
                                for dc in range(2):
                                    nc.tensor.matmul(
                                        pv[:],
                                        r(hT[:, dc, mc * 128:(mc + 1) * 128]),
                                        r(wqkv[:, dc, DQ * 2 + h * D:DQ * 2 + (h + 1) * D]),
                                        start=(dc == 0), stop=(dc == 1))
                                nc.vector.tensor_tensor(
                                    vh[:, mc, :].bitcast(F32R), pv[:],
                                    bqkv_vbc[:, h * D:(h + 1) * D], OP.add)
                            qa_h, ka_h = qa_tiles[h], ka_tiles[h]
                            for nh in range(2):
                                ns = slice(nh * 512, nh * 512 + 512)
                                av0 = ps_av.tile([128, 512], F32, tag="av")
                                av1 = ps_av.tile([128, 512], F32, tag="av")
                                for mc in range(8):
                                    sps = ps_s.tile([128, 512], F32, tag="s")
                                    nc.tensor.matmul(
                                        sps[:],
                                        r(ka_h[0:97, mc * 128:(mc + 1) * 128]),
                                        r(qa_h[0:97, ns]),
                                        start=True, stop=True)
                                    ex = ex_p.tile([128, 512], F32, tag="ex")
                                    nc.scalar.activation(ex[:].bitcast(F32R),
                                                         sps[:], AF.Exp,
                                                         scale=esc_t)
                                    nc.tensor.matmul(av0[:], r(vh[:, mc, 0:128]),
                                                     r(ex[:]), start=(mc == 0),
                                                     stop=(mc == 7))
                                    nc.tensor.matmul(av1[:], r(vh[:, mc, 128:256]),
                                                     r(ex[:]), start=(mc == 0),
                                                     stop=(mc == 7))
                                nc.vector.tensor_copy(
                                    outT[:, 2 * hh, ns].bitcast(F32R), av0[:])
                                nc.vector.tensor_copy(
                                    outT[:, 2 * hh + 1, ns].bitcast(F32R), av1[:])

                        # merge this group's heads (natural orientation only;
                        # x1^T comes from a PE transpose of x1_nat later)
                        for mc2 in range(8):
                            pn = ps_v.tile([128, D], F32, tag="v")
                            for c8 in range(8):
                                nc.tensor.matmul(
                                    pn[:],
                                    r(outT[:, c8, mc2 * 128:(mc2 + 1) * 128]),
                                    r(wm[:, g * 8 + c8, :]),
                                    start=(c8 == 0), stop=(c8 == 7))
                            if g == 0:
                                nc.vector.tensor_copy(accN[:, mc2, :], pn[:])
                            else:
                                nc.vector.tensor_tensor(accN[:, mc2, :],
                                                        accN[:, mc2, :],
                                                        pn[:], OP.add)

            # ---- phase D/E: residuals, time-conditioned FFN ----
            with tc.tile_pool(name="L3", bufs=1) as L3:
                xnat = L3.tile([128, 8, D], F32)
                for mc in range(8):
                    nc.sync.dma_start(xnat[:, mc, :],
                                        xn_d[mc * 128:(mc + 1) * 128, :])
                wf1 = L3.tile([128, 2, DE], F32)
                for dc in range(2):
                    nc.sync.dma_start(
                        wf1[:, dc, :].bitcast(F32R),
                        wf1_d[dc * 128:(dc + 1) * 128, :].bitcast(F32R))
                wf2 = L3.tile([128, 8, D], F32)
                for fc in range(8):
                    nc.sync.dma_start(
                        wf2[:, fc, :].bitcast(F32R),
                        wf2_d[fc * 128:(fc + 1) * 128, :].bitcast(F32R))
                sT = L3.tile([128, 2, N], F32)
                ffT = L3.tile([128, 8, N], F32)

                for dch in range(2):
                    # transpose raw proj_nat; x1^T = proj^T + bm + x^T fused
                    # into the PSUM->SBUF copy.  dch-major order lets the
                    # dc=0 norm start while dch=1 blocks still transpose.
                    for mc in range(8):
                        ptp = ps_v.tile([128, 128], F32, tag="v")
                        nc.tensor.transpose(
                            ptp[:], accN[:, mc, dch * 128:(dch + 1) * 128],
                            ident[:])
                        nc.vector.scalar_tensor_tensor(
                            out=accT[:, dch, mc * 128:(mc + 1) * 128],
                            in0=ptp[:], scalar=bm_t[:, dch:dch + 1],
                            in1=xT[:, dch, mc * 128:(mc + 1) * 128],
                            op0=OP.add, op1=OP.add)
                for dc in range(2):
                    # x2T = x1T + t_shift
                    nc.vector.tensor_scalar_add(accT[:, dc, :], accT[:, dc, :],
                                                ts_all[:, dc:dc + 1])
                    norm_affine(lambda: accT[:, dc, :], gamma1_t, beta1_t, dc)
                    act_silu(sT[:, dc, :].bitcast(F32R), accT[:, dc, :],
                             bias=nB[:, dc:dc + 1], scale=nA[:, dc:dc + 1])
                for mc in range(8):
                    # x1nat = proj + bm + xnat (only needed by the final
                    # residual; runs in the shadow of the FFN matmuls)
                    nc.vector.tensor_tensor(accN[:, mc, :], accN[:, mc, :],
                                            bm_bc[:], OP.add)
                    nc.vector.tensor_tensor(accN[:, mc, :], accN[:, mc, :],
                                            xnat[:, mc, :], OP.add)
                for fc in range(8):
                    for nh in range(2):
                        ns = slice(nh * 512, nh * 512 + 512)
                        pf = ps_s.tile([128, 512], F32, tag="s")
                        for dc in range(2):
                            nc.tensor.matmul(
                                pf[:], r(wf1[:, dc, fc * 128:(fc + 1) * 128]),
                                r(sT[:, dc, ns]), start=(dc == 0), stop=(dc == 1))
                        act_silu(ffT[:, fc, ns].bitcast(F32R), pf[:],
                                 bias=bf1sig[:, fc:fc + 1],
                                 scale=sig_all[:, fc:fc + 1], width=512)
                for mc in range(8):
                    po = ps_v.tile([128, D], F32, tag="v")
                    for fc in range(8):
                        nc.tensor.matmul(po[:],
                                         r(ffT[:, fc, mc * 128:(mc + 1) * 128]),
                                         r(wf2[:, fc, :]),
                                         start=(fc == 0), stop=(fc == 7))
                    osb = xnat[:, mc, :]
                    nc.vector.tensor_tensor(osb, po[:], accN[:, mc, :], OP.add)
                    nc.vector.tensor_tensor(osb, osb, bf2_bc[:], OP.add)
                    nc.sync.dma_start(out_d[mc * 128:(mc + 1) * 128, :], osb)

    nc.compile()
    return nc


def _get_nc():
    if "nc" not in _CACHE:
        _CACHE["nc"] = _build()
    return _CACHE["nc"]


def _make_in_maps(x, t, gamma, beta, pos_enc, Wqkv, bqkv, Wm, bm, Wt, bt,
                  gamma1, beta1, Wf1, bf1, Wf2, bf2, scale):
    x = np.asarray(x, np.float32)

    def _vecpack(t_b):
        rows = np.zeros((40, 128), np.float32)
        def put(i, v):
            rows[i:i + len(v) // 128] = np.reshape(v, (-1, 128))
        put(0, np.ravel(gamma)); put(2, np.ravel(beta)); put(4, np.ravel(bm))
        put(6, np.ravel(gamma1)); put(8, np.ravel(beta1)); put(10, t_b)
        # row 12 stays zero (fp32r free>=2 pad for the t matmul)
        put(13, np.ravel(bt)); put(23, np.ravel(bf1))
        put(31, np.ravel(bqkv)[:1024])
        rows[39] = -1.0 / (float(scale) * float(scale))
        return rows
    posT = np.ascontiguousarray(np.asarray(pos_enc, np.float32)[0].T)
    shared = {
        "posT": posT,
        "Wqkv": np.ascontiguousarray(Wqkv, np.float32),
        "Wm": np.ascontiguousarray(Wm, np.float32),
        "Wf1": np.ascontiguousarray(Wf1, np.float32),
        "Wf2": np.ascontiguousarray(Wf2, np.float32),
        "Wt": np.ascontiguousarray(Wt, np.float32),
        "bm": np.ascontiguousarray(np.ravel(bm), np.float32),
        "bqkv": np.ascontiguousarray(np.ravel(bqkv), np.float32),
        "bf2": np.ascontiguousarray(np.ravel(bf2), np.float32),
    }
    in_maps = []
    for b in range(NCORES):
        m = dict(shared)
        m["xT"] = np.ascontiguousarray(x[b].T)
        m["xnat"] = np.ascontiguousarray(x[b])
        m["vecpack"] = _vecpack(np.ravel(np.asarray(t, np.float32)[b]))
        in_maps.append(m)
    return in_maps


def kernel(**inputs):
    from concourse.bass_utils import run_bass_kernel_spmd

    nc = _get_nc()
    in_maps = _make_in_maps(**inputs)
    res = run_bass_kernel_spmd(nc, in_maps, list(range(NCORES)))
    return np.stack([res.results[i]["out"] for i in range(NCORES)], axis=0)


# revision 61
# speedup vs baseline: 1.0004x; 1.0004x over previous
"""Trainium2 Bass kernel for nn_AttentionWithTime.

Data-parallel over batch: B=8 batches -> 8 NeuronCores, one batch each.
Per-core strategy: all sequence-dim normalization runs in transposed
([feature, seq]) orientation so it is a free-dim reduction; the
Gaussian-kernel attention distance matrix d2 = q2 + k2 - 2qk is produced by
a single K=97 augmented f32r matmul (k rows | k2 | zero pad | ones against
-2q rows | ones | zero pad | q2 -- aug rows sit at partitions 64/96 because
engine partition offsets must be multiples of 32), exp'd in one ScalarE
pass from PSUM; attention output accumulates head-transposed, is merged
through Wm in natural orientation only, and x1^T is recovered with 16 PE
transposes whose PSUM->SBUF copies fuse the bm + x residual.  Small
per-feature vectors ship as one packed [40,128] DMA + a single PE
transpose.  All matmuls run as float32r (full PE rate at free>=256).
"""

import numpy as np

B, N, D = 8, 1024, 256
H, DH = 8, 64
DE, DT = 1024, 256
DQ = H * DH            # 512
DV = D * H             # 2048
DQKV = 2 * DQ + DV     # 3072
EPS = 1e-5
NCORES = 8

_CACHE = {}
SIM_COMPAT = False  # CoreSim lacks Silu; decompose when simulating


def _build():
    import concourse.bass as bass
    import concourse.mybir as mybir
    import concourse.tile as tile
    from concourse import bacc
    from concourse.masks import make_identity

    F32 = mybir.dt.float32
    F32R = mybir.dt.float32r
    AF = mybir.ActivationFunctionType
    OP = mybir.AluOpType

    nc = bacc.Bacc()

    xT_d = nc.declare_dram_parameter("xT", [D, N], F32, isOutput=False)
    xn_d = nc.declare_dram_parameter("xnat", [N, D], F32, isOutput=False)
    posT_d = nc.declare_dram_parameter("posT", [D, N], F32, isOutput=False)
    wqkv_d = nc.declare_dram_parameter("Wqkv", [D, DQKV], F32, isOutput=False)
    wm_d = nc.declare_dram_parameter("Wm", [DV, D], F32, isOutput=False)
    wf1_d = nc.declare_dram_parameter("Wf1", [D, DE], F32, isOutput=False)
    wf2_d = nc.declare_dram_parameter("Wf2", [DE, D], F32, isOutput=False)
    wt_d = nc.declare_dram_parameter("Wt", [DT, D + DE], F32, isOutput=False)
    bm_d = nc.declare_dram_parameter("bm", [D], F32, isOutput=False)
    bqkv_d = nc.declare_dram_parameter("bqkv", [DQKV], F32, isOutput=False)
    bf2_d = nc.declare_dram_parameter("bf2", [D], F32, isOutput=False)
    vp_d = nc.declare_dram_parameter("vecpack", [40, 128], F32, isOutput=False)
    out_d = nc.declare_dram_parameter("out", [N, D], F32, isOutput=True)

    def r(ap):
        return ap.bitcast(F32R)

    with tile.TileContext(nc) as tc:
        with (
            tc.tile_pool(name="L1", bufs=1) as L1,
            tc.tile_pool(name="ps_s", bufs=3, space="PSUM") as ps_s,
            tc.tile_pool(name="ps_av", bufs=3, space="PSUM") as ps_av,
            tc.tile_pool(name="ps_v", bufs=2, space="PSUM") as ps_v,
        ):
            # ---- long-lived SBUF ----
            xT = L1.tile([128, 2, N], F32)
            hT = L1.tile([128, 2, N], F32)
            accT = L1.tile([128, 2, N], F32)
            accN = L1.tile([128, 8, D], F32)
            wm = L1.tile([128, 16, D], F32)
            for dc in range(2):
                nc.sync.dma_start(xT[:, dc, :], xT_d[dc * 128:(dc + 1) * 128, :])
            # small per-feature vectors arrive packed as [40,128]; one PE
            # transpose turns them into per-partition columns.
            vp_s = L1.tile([40, 128], F32)
            nc.sync.dma_start(vp_s[:], vp_d[:])
            ident = L1.tile([128, 128], F32)
            make_identity(nc, ident[:])
            vp = L1.tile([128, 40], F32)
            pvp = ps_v.tile([128, 40], F32, tag="v")
            nc.tensor.transpose(pvp[:], vp_s[:], ident[0:40, 0:40])
            nc.vector.tensor_copy(vp[:], pvp[:])
            # only the t-vector columns feed a matmul -> separate f32r tile
            tvec_t = L1.tile([128, 3], F32)
            nc.vector.tensor_copy(tvec_t[:].bitcast(F32R), pvp[:, 10:13])

            # vecpack column map (see _make_in_maps): gamma 0:2, beta 2:4,
            # bm 4:6, gamma1 6:8, beta1 8:10, tvec 10:12 (+zero col 12),
            # bt 13:23, bf1 23:31, bqkv q/k chunks 31:39, expscale 39
            gamma_t = vp[:, 0:2]
            beta_t = vp[:, 2:4]
            bm_t = vp[:, 4:6]
            gamma1_t = vp[:, 6:8]
            beta1_t = vp[:, 8:10]
            bt_t = vp[:, 13:23]
            bf1_t = vp[:, 23:31]
            bqkv_q = vp[:, 31:39]
            esc_t = vp[:, 39:40]
            bm_bc = L1.tile([128, D], F32)
            bf2_bc = L1.tile([128, D], F32)
            eps_t = L1.tile([128, 1], F32)
            nc.vector.memset(eps_t[:], EPS)
            # f32r cannot be memset directly; stage in f32 then cast-copy
            oS = L1.tile([128, 2], F32)
            nc.vector.memset(oS[:], 0.0)
            nc.vector.memset(oS[0:64, 0:1], 1.0)
            nc.vector.memset(oS[64:128, 1:2], 1.0)
            onesA = L1.tile([128, 1], F32)
            nc.vector.tensor_copy(onesA[:].bitcast(F32R), oS[:, 0:1])
            onesB = L1.tile([128, 1], F32)
            nc.vector.tensor_copy(onesB[:].bitcast(F32R), oS[:, 1:2])
            # staging rows for augmented tiles: kq_c = [1; 0*32], kk_c = [0*32; 1]
            kq_c = L1.tile([33, N], F32)
            nc.vector.memset(kq_c[0:33, :], 0.0)
            nc.vector.memset(kq_c[0:1, :], 1.0)
            kk_c = L1.tile([33, N], F32)
            nc.vector.memset(kk_c[0:33, :], 0.0)
            nc.vector.memset(kk_c[32:33, :], 1.0)
            ts_all = L1.tile([128, 10], F32)
            sig_all = L1.tile([128, 8], F32)
            bf1sig = L1.tile([128, 8], F32)
            stat = L1.tile([128, 2, 2, nc.vector.BN_STATS_DIM], F32)
            mv = L1.tile([128, 2, nc.vector.BN_AGGR_DIM], F32)
            nA = L1.tile([128, 2], F32)
            nB = L1.tile([128, 2], F32)
            tmp1 = L1.tile([128, 2], F32)

            if SIM_COMPAT:
                sil_t = L1.tile([128, N], F32, tag="sil")
            else:
                sil_t = None

            def act_silu(out_ap, in_ap, bias=0.0, scale=1.0, width=N):
                # out = Silu(in*scale + bias); decomposed for CoreSim
                if not SIM_COMPAT:
                    nc.scalar.activation(out_ap, in_ap, AF.Silu,
                                         bias=bias, scale=scale)
                    return
                sg = sil_t
                nc.scalar.activation(sg[:, :width], in_ap, AF.Sigmoid,
                                     bias=bias, scale=scale)
                if isinstance(scale, float) and isinstance(bias, float):
                    if scale == 1.0 and bias == 0.0:
                        nc.vector.tensor_mul(out_ap, in_ap, sg[:, :width])
                        return
                    raise NotImplementedError
                nc.vector.tensor_scalar(out_ap, in_ap, scale, bias,
                                        OP.mult, OP.add)
                nc.vector.tensor_mul(out_ap, out_ap, sg[:, :width])

            def norm_affine(src_ap_fn, g_t, b_t, dc):
                # src [128, N] in transposed layout; per-partition affine out
                nc.vector.bn_stats(stat[:, dc, 0, :], src_ap_fn()[:, 0:512])
                nc.vector.bn_stats(stat[:, dc, 1, :], src_ap_fn()[:, 512:1024])
                nc.vector.bn_aggr(mv[:, dc, :], stat[:, dc, :, :])
                nc.scalar.activation(tmp1[:, dc:dc + 1], mv[:, dc, 1:2],
                                     AF.Sqrt, bias=eps_t[:], scale=1.0)
                nc.vector.reciprocal(nA[:, dc:dc + 1], tmp1[:, dc:dc + 1])
                nc.vector.tensor_mul(nA[:, dc:dc + 1], nA[:, dc:dc + 1],
                                     g_t[:, dc:dc + 1])
                nc.vector.tensor_mul(tmp1[:, dc:dc + 1], mv[:, dc, 0:1],
                                     nA[:, dc:dc + 1])
                nc.vector.tensor_tensor(nB[:, dc:dc + 1], b_t[:, dc:dc + 1],
                                        tmp1[:, dc:dc + 1], OP.subtract)

            with tc.tile_pool(name="L2", bufs=1) as L2:
                # load order matters: q/k weight columns + posT gate the
                # prologue; v columns, Wm and broadcast consts come later.
                wqkv = L2.tile([128, 2, DQKV], F32)
                posT = L2.tile([128, 2, N], F32)
                wt = L2.tile([128, 2, D + DE], F32)
                bqkv_vbc = L2.tile([128, DV], F32)
                for dc in range(2):
                    nc.sync.dma_start(
                        wqkv[:, dc, 0:2 * DQ].bitcast(F32R),
                        wqkv_d[dc * 128:(dc + 1) * 128, 0:2 * DQ].bitcast(F32R))
                for dc in range(2):
                    nc.sync.dma_start(posT[:, dc, :],
                                        posT_d[dc * 128:(dc + 1) * 128, :])
                for dc in range(2):
                    nc.scalar.dma_start(
                        wt[:, dc, :].bitcast(F32R),
                        wt_d[dc * 128:(dc + 1) * 128, :].bitcast(F32R))
                for dc in range(2):
                    nc.sync.dma_start(
                        wqkv[:, dc, 2 * DQ:].bitcast(F32R),
                        wqkv_d[dc * 128:(dc + 1) * 128, 2 * DQ:].bitcast(F32R))
                nc.scalar.dma_start(bqkv_vbc[:],
                                    bqkv_d[2 * DQ:].partition_broadcast(128))
                for fc in range(16):
                    nc.sync.dma_start(
                        wm[:, fc, :].bitcast(F32R),
                        wm_d[fc * 128:(fc + 1) * 128, :].bitcast(F32R))
                nc.scalar.dma_start(bm_bc[:], bm_d[:].partition_broadcast(128))
                nc.scalar.dma_start(bf2_bc[:], bf2_d[:].partition_broadcast(128))

                # ---- phase A: feature_norm + pos_enc + swish -> hT ----
                for dc in range(2):
                    norm_affine(lambda: xT[:, dc, :], gamma_t, beta_t, dc)
                    for nh in range(2):
                        ns = slice(nh * 512, nh * 512 + 512)
                        nc.vector.tensor_scalar(hT[:, dc, ns].bitcast(F32R),
                                                xT[:, dc, ns],
                                                nA[:, dc:dc + 1],
                                                nB[:, dc:dc + 1],
                                                OP.mult, OP.add)
                        nc.vector.tensor_tensor(hT[:, dc, ns].bitcast(F32R),
                                                hT[:, dc, ns],
                                                posT[:, dc, ns], OP.add)
                        act_silu(hT[:, dc, ns].bitcast(F32R), hT[:, dc, ns],
                                 width=512)

                # ---- phase A2: time branch ts = t @ Wt + bt ----
                for j in range(10):
                    # fp32r matmul needs free>=2: col 1 accumulates junk
                    # (t1-projection then +0), only col 0 is consumed.
                    pts = ps_v.tile([128, 2], F32, tag="v")
                    for dc in range(2):
                        nc.tensor.matmul(pts[:], r(wt[:, dc, j * 128:(j + 1) * 128]),
                                         r(tvec_t[:, dc:dc + 2]),
                                         start=(dc == 0), stop=(dc == 1))
                    nc.vector.tensor_tensor(ts_all[:, j:j + 1], pts[:, 0:1],
                                            bt_t[:, j:j + 1], OP.add)
                for j in range(8):
                    nc.scalar.activation(sig_all[:, j:j + 1],
                                         ts_all[:, j + 2:j + 3], AF.Sigmoid)
                    nc.vector.tensor_mul(bf1sig[:, j:j + 1], sig_all[:, j:j + 1],
                                         bf1_t[:, j:j + 1])

                with (
                    tc.tile_pool(name="qa_p", bufs=4) as qa_p,
                    tc.tile_pool(name="ka_p", bufs=4) as ka_p,
                    tc.tile_pool(name="sq_p", bufs=3) as sq_p,
                    tc.tile_pool(name="vh_p", bufs=2) as vh_p,
                    tc.tile_pool(name="ex_p", bufs=4) as ex_p,
                    tc.tile_pool(name="oT_p", bufs=1) as oT_p,
                ):
                    qa_tiles, ka_tiles = {}, {}

                    def project_qk(j):
                        # heads 2j, 2j+1 -> augmented q/k tiles.
                        # aug layout (rows): 0:64 data, 64 = ones(q)/k2(k),
                        # 65:96 zero, 96 = q2(q)/ones(k); engine partition
                        # offsets must be multiples of 32.
                        h0, h1 = 2 * j, 2 * j + 1
                        qa0 = qa_p.tile([97, N], F32, tag="qa")
                        qa1 = qa_p.tile([97, N], F32, tag="qa")
                        ka0 = ka_p.tile([97, N], F32, tag="ka")
                        ka1 = ka_p.tile([97, N], F32, tag="ka")
                        qa_tiles[h0], qa_tiles[h1] = qa0, qa1
                        ka_tiles[h0], ka_tiles[h1] = ka0, ka1
                        for qa in (qa0, qa1):
                            nc.gpsimd.tensor_copy(qa[64:97, :].bitcast(F32R),
                                                  kq_c[0:33, :])
                        for ka in (ka0, ka1):
                            nc.gpsimd.tensor_copy(ka[64:97, :].bitcast(F32R),
                                                  kk_c[0:33, :])
                        for nh in range(2):
                            ns = slice(nh * 512, nh * 512 + 512)
                            pq = ps_s.tile([128, 512], F32, tag="s")
                            pk = ps_s.tile([128, 512], F32, tag="s")
                            for dc in range(2):
                                nc.tensor.matmul(
                                    pq[:], r(wqkv[:, dc, j * 128:(j + 1) * 128]),
                                    r(hT[:, dc, ns]),
                                    start=(dc == 0), stop=(dc == 1))
                            for dc in range(2):
                                nc.tensor.matmul(
                                    pk[:], r(wqkv[:, dc, DQ + j * 128:DQ + (j + 1) * 128]),
                                    r(hT[:, dc, ns]),
                                    start=(dc == 0), stop=(dc == 1))
                            # q rows: -2*(q+bias); k rows: k+bias; biases
                            # folded into the copies and the squares
                            bq, bk = bqkv_q[:, j:j + 1], bqkv_q[:, 4 + j:5 + j]
                            nc.vector.tensor_scalar(
                                qa0[0:64, ns].bitcast(F32R), pq[0:64, :],
                                bq[0:64, :], -2.0, OP.add, OP.mult)
                            nc.vector.tensor_scalar(
                                qa1[0:64, ns].bitcast(F32R), pq[64:128, :],
                                bq[64:128, :], -2.0, OP.add, OP.mult)
                            nc.vector.tensor_scalar(
                                ka0[0:64, ns].bitcast(F32R), pk[0:64, :],
                                bk[0:64, :], None, OP.add)
                            nc.vector.tensor_scalar(
                                ka1[0:64, ns].bitcast(F32R), pk[64:128, :],
                                bk[64:128, :], None, OP.add)
                            sqq = sq_p.tile([128, 512], F32, tag="sq")
                            sqk = sq_p.tile([128, 512], F32, tag="sq")
                            nc.scalar.activation(sqq[:].bitcast(F32R), pq[:],
                                                 AF.Square, bias=bq)
                            nc.scalar.activation(sqk[:].bitcast(F32R), pk[:],
                                                 AF.Square, bias=bk)
                            for (aug, row, ones_v, sq_t) in (
                                (qa0, 96, onesA, sqq), (qa1, 96, onesB, sqq),
                                (ka0, 64, onesA, sqk), (ka1, 64, onesB, sqk),
                            ):
                                p2 = ps_av.tile([1, 512], F32, tag="av")
                                nc.tensor.matmul(p2[:], r(ones_v[:]), r(sq_t[:]),
                                                 start=True, stop=True)
                                nc.scalar.activation(
                                    aug[row:row + 1, ns].bitcast(F32R), p2[0:1, :],
                                    AF.Copy)

                    # ---- attention + split merge, two groups of 4 heads;
                    # projections prefetched one head-pair ahead ----
                    project_qk(0)
                    for g in range(2):
                        outT = oT_p.tile([128, 8, N], F32, tag="outT")
                        for hh in range(4):
                            h = 4 * g + hh
                            if h % 2 == 0 and h < 6:
                                project_qk(h // 2 + 1)
                            vh = vh_p.tile([128, 8, D], F32, tag="vh")
                            for mc in range(8):
                                pv = ps_v.tile([128, D], F32, tag="v")
                                for dc in range(2):
                                    nc.tensor.matmul(
                                        pv[:],
                                        r(hT[:, dc, mc * 128:(mc + 1) * 128]),
                                        r(wqkv[:, dc, DQ * 2 + h * D:DQ * 2 + (h + 1) * D]),
                                        start=(dc == 0), stop=(dc == 1))
                                nc.vector.tensor_tensor(
                                    vh[:, mc, :].bitcast(F32R), pv[:],
                                    bqkv_vbc[:, h * D:(h + 1) * D], OP.add)
                            qa_h, ka_h = qa_tiles[h], ka_tiles[h]
                            for nh in range(2):
                                ns = slice(nh * 512, nh * 512 + 512)
                                av0 = ps_av.tile([128, 512], F32, tag="av")
                                av1 = ps_av.tile([128, 512], F32, tag="av")
                                for mc in range(8):
                                    sps = ps_s.tile([128, 512], F32, tag="s")
                                    nc.tensor.matmul(
                                        sps[:],
                                        r(ka_h[0:97, mc * 128:(mc + 1) * 128]),
                                        r(qa_h[0:97, ns]),
                                        start=True, stop=True)
                                    ex = ex_p.tile([128, 512], F32, tag="ex")
                                    nc.scalar.activation(ex[:].bitcast(F32R),
                                                         sps[:], AF.Exp,
                                                         scale=esc_t)
                                    nc.tensor.matmul(av0[:], r(vh[:, mc, 0:128]),
                                                     r(ex[:]), start=(mc == 0),
                                                     stop=(mc == 7))
                                    nc.tensor.matmul(av1[:], r(vh[:, mc, 128:256]),
                                                     r(ex[:]), start=(mc == 0),
                                                     stop=(mc == 7))
                                nc.vector.tensor_copy(
                                    outT[:, 2 * hh, ns].bitcast(F32R), av0[:])
                                nc.vector.tensor_copy(
                                    outT[:, 2 * hh + 1, ns].bitcast(F32R), av1[:])

                        # merge this group's heads (natural orientation only;
                        # x1^T comes from a PE transpose of x1_nat later)
                        for mc2 in range(8):
                            pn = ps_v.tile([128, D], F32, tag="v")
                            for c8 in range(8):
                                nc.tensor.matmul(
                                    pn[:],
                                    r(outT[:, c8, mc2 * 128:(mc2 + 1) * 128]),
                                    r(wm[:, g * 8 + c8, :]),
                                    start=(c8 == 0), stop=(c8 == 7))
                            if g == 0:
                                nc.vector.tensor_copy(accN[:, mc2, :], pn[:])
                            else:
                                nc.vector.tensor_tensor(accN[:, mc2, :],
                                                        accN[:, mc2, :],
                                                        pn[:], OP.add)

            # ---- phase D/E: residuals, time-conditioned FFN ----
            with tc.tile_pool(name="L3", bufs=1) as L3:
                xnat = L3.tile([128, 8, D], F32)
                for mc in range(8):
                    nc.sync.dma_start(xnat[:, mc, :],
                                        xn_d[mc * 128:(mc + 1) * 128, :])
                wf1 = L3.tile([128, 2, DE], F32)
                for dc in range(2):
                    nc.sync.dma_start(
                        wf1[:, dc, :].bitcast(F32R),
                        wf1_d[dc * 128:(dc + 1) * 128, :].bitcast(F32R))
                wf2 = L3.tile([128, 8, D], F32)
                for fc in range(8):
                    nc.sync.dma_start(
                        wf2[:, fc, :].bitcast(F32R),
                        wf2_d[fc * 128:(fc + 1) * 128, :].bitcast(F32R))
                sT = L3.tile([128, 2, N], F32)
                ffT = L3.tile([128, 8, N], F32)

                for dch in range(2):
                    # transpose raw proj_nat; x1^T = proj^T + bm + x^T fused
                    # into the PSUM->SBUF copy.  dch-major order lets the
                    # dc=0 norm start while dch=1 blocks still transpose.
                    for mc in range(8):
                        ptp = ps_v.tile([128, 128], F32, tag="v")
                        nc.tensor.transpose(
                            ptp[:], accN[:, mc, dch * 128:(dch + 1) * 128],
                            ident[:])
                        nc.vector.scalar_tensor_tensor(
                            out=accT[:, dch, mc * 128:(mc + 1) * 128],
                            in0=ptp[:], scalar=bm_t[:, dch:dch + 1],
                            in1=xT[:, dch, mc * 128:(mc + 1) * 128],
                            op0=OP.add, op1=OP.add)
                for dc in range(2):
                    # x2T = x1T + t_shift
                    nc.vector.tensor_scalar_add(accT[:, dc, :], accT[:, dc, :],
                                                ts_all[:, dc:dc + 1])
                    norm_affine(lambda: accT[:, dc, :], gamma1_t, beta1_t, dc)
                    act_silu(sT[:, dc, :].bitcast(F32R), accT[:, dc, :],
                             bias=nB[:, dc:dc + 1], scale=nA[:, dc:dc + 1])
                for mc in range(8):
                    # x1nat = proj + bm + xnat (only needed by the final
                    # residual; runs in the shadow of the FFN matmuls)
                    nc.vector.tensor_tensor(accN[:, mc, :], accN[:, mc, :],
                                            bm_bc[:], OP.add)
                    nc.vector.tensor_tensor(accN[:, mc, :], accN[:, mc, :],
                                            xnat[:, mc, :], OP.add)
                for fc in range(8):
                    for nh in range(2):
                        ns = slice(nh * 512, nh * 512 + 512)
                        pf = ps_s.tile([128, 512], F32, tag="s")
                        for dc in range(2):
                            nc.tensor.matmul(
                                pf[:], r(wf1[:, dc, fc * 128:(fc + 1) * 128]),
                                r(sT[:, dc, ns]), start=(dc == 0), stop=(dc == 1))
                        act_silu(ffT[:, fc, ns].bitcast(F32R), pf[:],
                                 bias=bf1sig[:, fc:fc + 1],
                                 scale=sig_all[:, fc:fc + 1], width=512)
                for mc in range(8):
                    po = ps_v.tile([128, D], F32, tag="v")
                    for fc in range(8):
                        nc.tensor.matmul(po[:],
                                         r(ffT[:, fc, mc * 128:(mc + 1) * 128]),
                                         r(wf2[:, fc, :]),
                                         start=(fc == 0), stop=(fc == 7))
                    osb = xnat[:, mc, :]
                    nc.vector.tensor_tensor(osb, po[:], accN[:, mc, :], OP.add)
                    nc.vector.tensor_tensor(osb, osb, bf2_bc[:], OP.add)
                    nc.sync.dma_start(out_d[mc * 128:(mc + 1) * 128, :], osb)

    nc.compile()
    return nc


def _get_nc():
    if "nc" not in _CACHE:
        _CACHE["nc"] = _build()
    return _CACHE["nc"]


def _make_in_maps(x, t, gamma, beta, pos_enc, Wqkv, bqkv, Wm, bm, Wt, bt,
                  gamma1, beta1, Wf1, bf1, Wf2, bf2, scale):
    x = np.asarray(x, np.float32)

    def _vecpack(t_b):
        rows = np.zeros((40, 128), np.float32)
        def put(i, v):
            rows[i:i + len(v) // 128] = np.reshape(v, (-1, 128))
        put(0, np.ravel(gamma)); put(2, np.ravel(beta)); put(4, np.ravel(bm))
        put(6, np.ravel(gamma1)); put(8, np.ravel(beta1)); put(10, t_b)
        # row 12 stays zero (fp32r free>=2 pad for the t matmul)
        put(13, np.ravel(bt)); put(23, np.ravel(bf1))
        put(31, np.ravel(bqkv)[:1024])
        rows[39] = -1.0 / (float(scale) * float(scale))
        return rows
    posT = np.ascontiguousarray(np.asarray(pos_enc, np.float32)[0].T)
    shared = {
        "posT": posT,
        "Wqkv": np.ascontiguousarray(Wqkv, np.float32),
        "Wm": np.ascontiguousarray(Wm, np.float32),
        "Wf1": np.ascontiguousarray(Wf1, np.float32),
        "Wf2": np.ascontiguousarray(Wf2, np.float32),
        "Wt": np.ascontiguousarray(Wt, np.float32),
        "bm": np.ascontiguousarray(np.ravel(bm), np.float32),
        "bqkv": np.ascontiguousarray(np.ravel(bqkv), np.float32),
        "bf2": np.ascontiguousarray(np.ravel(bf2), np.float32),
    }
    in_maps = []
    for b in range(NCORES):
        m = dict(shared)
        m["xT"] = np.ascontiguousarray(x[b].T)
        m["xnat"] = np.ascontiguousarray(x[b])
        m["vecpack"] = _vecpack(np.ravel(np.asarray(t, np.float32)[b]))
        in_maps.append(m)
    return in_maps


def kernel(**inputs):
    from concourse.bass_utils import run_bass_kernel_spmd

    nc = _get_nc()
    in_maps = _make_in_maps(**inputs)
    res = run_bass_kernel_spmd(nc, in_maps, list(range(NCORES)))
    return np.stack([res.results[i]["out"] for i in range(NCORES)], axis=0)
